# revision 13
# baseline (speedup 1.0000x reference)
"""Expert-parallel MoE (DBRX-style SwiGLU FFN) on 8 TRN2 NeuronCores.

Strategy: experts are paired by token count (biggest with smallest) and each
pair is split across 2 cores along the FFN (F) axis: core 2p takes F-rows
[0:1024) of both experts in pair p, core 2p+1 takes F-rows [1024:2048).
Each core therefore runs TWO half-width SwiGLU FFNs (slot A = the pair's
bigger expert, slot B = the smaller), and the host sums the two cores'
partial y contributions. This balances PE work across cores at zero extra
weight traffic: per-core tokens go from max_e n_e (503 for seed 0) to
~(n_a+n_b)/2 (~492).

Per slot, in "transposed activation" form (all bf16, f32 PSUM):

    up^T   = w_up^T  @ x^T    (K = H = 1024, 8 K-subtiles)
    gate^T = w_gate^T @ x^T
    h^T    = silu(up^T) * gate^T
    y^T    = w_down^T @ h^T   (K = F/2 = 1024, 8 K-subtiles)

Schedule notes (from NTFF profile analysis):
  - ~6.2 us framework preamble before any user instruction.
  - DMA kicks are issued from two HWDGE engines in parallel: weights on
    Sync, x / outputs on Scalar, so first data lands ~2 us earlier than
    a single-engine kick chain.
  - up+gate weights for one 128-row F-chunk live in ONE dram tensor,
    kicked as two half DMAs so the up-half (needed first) lands sooner.
  - Junk matmuls cover preamble->first-data and ungate the HAM PE clock
    (1.2 -> 2.4 GHz after ~3.4 us of sustained activity).
  - The final output chunk is split fine (…/128/64/32) so the last
    PSUM-drain + DMA-out chain after the very last matmul is short.
"""

import numpy as np
import ml_dtypes

import concourse.bacc as bacc
import concourse.mybir as mybir
import concourse.tile as tile
from concourse import bass_utils

HIDDEN = 1024
FFN = 2048
FH = FFN // 2        # F-half per core
N_EXPERTS = 8
N_PAIRS = 4
P = 128
KO_H = HIDDEN // P   # 8  K-subtiles for up/gate (K = H)
FC_N = FH // P       # 8  F-chunks per slot (output partition tiles of stage A)
KO_F = FH // P       # 8  K-subtiles for down (K = F-half)
HC_N = HIDDEN // P   # 8  H-chunks (output partition tiles of stage B)

WARM_N = 72          # junk matmuls (N=128) covering preamble->first-data:
                     # ~32 run cold (107 ns) until HAM ungates at ~10.4 us,
                     # the rest warm (56 ns), ending ~12.8 us when the
                     # critical 1.5 MB DMA prefix (xa + fc0 weights) has
                     # fully landed (bytes + ~1.5 us HWDGE sem receipt)

BF16 = ml_dtypes.bfloat16

_compiled = {}  # (cap_a, cap_b) -> compiled Bacc module


def _tail_split(nt):
    """Halve the final token chunk once: the last drain+DMA chain after the
    very last matmul is short, without serializing several kick rounds."""
    if nt <= 256:
        return [nt]
    return [256, nt - 256]


def _build(cap_a, cap_b):
    f32 = mybir.dt.float32
    bf16 = mybir.dt.bfloat16

    nc = bacc.Bacc("TRN2", debug=False, enable_asserts=False,
                   num_devices=N_EXPERTS)
    xa_d = nc.dram_tensor("xa", [P, KO_H, cap_a], bf16, kind="ExternalInput")
    xb_d = nc.dram_tensor("xb", [P, KO_H, cap_b], bf16, kind="ExternalInput")
    uga_d = nc.dram_tensor("uga", [FC_N, P, 2, KO_H, P], bf16,
                           kind="ExternalInput")
    ugb_d = nc.dram_tensor("ugb", [FC_N, P, 2, KO_H, P], bf16,
                           kind="ExternalInput")
    wda_d = nc.dram_tensor("wda", [HC_N, P, KO_F, P], bf16,
                           kind="ExternalInput")
    wdb_d = nc.dram_tensor("wdb", [HC_N, P, KO_F, P], bf16,
                           kind="ExternalInput")
    ya_d = nc.dram_tensor("ya", [HC_N, P, cap_a], bf16, kind="ExternalOutput")
    yb_d = nc.dram_tensor("yb", [HC_N, P, cap_b], bf16, kind="ExternalOutput")

    with tile.TileContext(nc) as tc:
        with (
            tc.tile_pool(name="persist", bufs=1) as persist,
            tc.tile_pool(name="wpool", bufs=6) as wpool,
            tc.tile_pool(name="spool", bufs=4) as spool,
            tc.tile_pool(name="psum", bufs=2, space="PSUM") as psum,
        ):
            # PE clock warm-up: junk matmuls keep the PE busy from the end
            # of the engine preamble until the first real inputs land.
            warm = persist.tile([P, P], bf16, tag="warm")
            nc.vector.memset(warm[:], 0)
            pwarm = psum.tile([P, 512], f32, tag="pwarm", name="pwarm")
            for _ in range(WARM_N):
                nc.tensor.matmul(pwarm[:, :P], warm, warm, start=True,
                                 stop=True)

            xa = persist.tile([P, KO_H, cap_a], bf16, tag="xa")
            xb = persist.tile([P, KO_H, cap_b], bf16, tag="xb")
            hta = persist.tile([P, KO_F, cap_a], bf16, tag="hta")
            htb = persist.tile([P, KO_F, cap_b], bf16, tag="htb")

            # Early DMA bandwidth is zero-sum across queues: the first real
            # matmul chain needs exactly [xa ko0-1, ug0-up, ug0-gate], so
            # those three transfers go SERIALLY FIRST on Sync's queue, then
            # the remaining xa quarters. xb is kicked from Scalar's queue
            # mid-stage-A (emitted in the fc loop below) when the critical
            # window has passed.
            nc.sync.dma_start(xa[:, 0:2], xa_d.ap()[:, 0:2])

            slots = (
                (xa, hta, uga_d, wda_d, ya_d, cap_a),
                (xb, htb, ugb_d, wdb_d, yb_d, cap_b),
            )
            for s, (x_t, ht, ug_d, wd_d, y_d, cap) in enumerate(slots):
                tchunks = [(t0, min(512, cap - t0))
                           for t0 in range(0, cap, 512)]

                # Stage A: h^T[fc] = silu(up^T) * gate^T per 128-row F-chunk
                for fc in range(FC_N):
                    ug_t = wpool.tile([P, 2, KO_H, P], bf16, tag="ug")
                    # up half first (pu chain leads), then gate half
                    nc.sync.dma_start(ug_t[:, 0], ug_d.ap()[fc][:, 0])
                    nc.sync.dma_start(ug_t[:, 1], ug_d.ap()[fc][:, 1])
                    if s == 0 and fc == 0:
                        # rest of xa, behind the fc0 weights on the same queue
                        for q in range(1, 4):
                            nc.sync.dma_start(xa[:, 2 * q:2 * q + 2],
                                              xa_d.ap()[:, 2 * q:2 * q + 2])
                    for t0, nt in tchunks:
                        pu = psum.tile([P, 512], f32, tag="pu",
                                       name="pu")[:, :nt]
                        pg = psum.tile([P, 512], f32, tag="pg",
                                       name="pg")[:, :nt]
                        # interleave the two accumulation chains, pu leading
                        # pg by one ko: x quarters and the up weight half are
                        # consumed as soon as they land
                        seq = []
                        for ko in range(KO_H):
                            seq.append((pu, 0, ko))
                            if ko >= 1:
                                seq.append((pg, 1, ko - 1))
                        seq.append((pg, 1, KO_H - 1))
                        for dst, ug_i, ko in seq:
                            nc.tensor.matmul(dst, ug_t[:, ug_i, ko],
                                             x_t[:, ko, t0:t0 + nt],
                                             start=(ko == 0),
                                             stop=(ko == KO_H - 1))
                        su = spool.tile([P, 512], f32, tag="silu",
                                        name="su")[:, :nt]
                        nc.scalar.activation(
                            su, pu, mybir.ActivationFunctionType.Sigmoid)
                        nc.vector.tensor_mul(su, su, pu)
                        nc.vector.tensor_mul(ht[:, fc, t0:t0 + nt], su, pg)
                    if s == 0 and fc == 1:
                        # xb streams on Scalar's queue once the early window
                        # is past; needed only at slot B (~40 us later). The
                        # dummy 1-col copy creates a data dependency on fc1's
                        # output so the scheduler cannot hoist the 1 MB xb
                        # transfer into the critical first-weights window
                        # (it would starve that stream of DMA bandwidth).
                        nc.vector.tensor_copy(xb[:, 0, 0:1], ht[:, 1, 0:1])
                        nc.scalar.dma_start(xb[:], xb_d.ap())

                # Stage B: y^T[hc] = w_down^T @ h^T per 128-row H-chunk
                for hc in range(HC_N):
                    wd_t = wpool.tile([P, KO_F, P], bf16, tag="wd")
                    nc.sync.dma_start(wd_t[:], wd_d.ap()[hc])
                    chunks = tchunks
                    if s == len(slots) - 1 and hc == HC_N - 1:
                        t0l, ntl = tchunks[-1]
                        chunks = list(tchunks[:-1])
                        for sz in _tail_split(ntl):
                            chunks.append((t0l, sz))
                            t0l += sz
                    for ci, (t0, nt) in enumerate(chunks):
                        py = psum.tile([P, 512], f32, tag="py",
                                       name="py")[:, :nt]
                        for ko in range(KO_F):
                            nc.tensor.matmul(py, wd_t[:, ko],
                                             ht[:, ko, t0:t0 + nt],
                                             start=(ko == 0),
                                             stop=(ko == KO_F - 1))
                        yo = spool.tile([P, 512], bf16, tag="yo",
                                        name="yo")[:, :nt]
                        nc.vector.tensor_copy(yo, py)
                        # the very last chunks alternate kick engines so the
                        # final two output DMAs issue in parallel, not FIFO
                        last_hc = (s == len(slots) - 1 and hc == HC_N - 1)
                        eng = nc.sync if (last_hc and ci == len(chunks) - 1) \
                            else nc.scalar
                        eng.dma_start(y_d.ap()[hc][:, t0:t0 + nt], yo)

    nc.compile()
    return nc


def _roundup(v, m):
    return max(((v + m - 1) // m) * m, 64)


def _prepare(x, top_weights, top_experts):
    """Host routing: unique tokens + combine weight per expert, pairing."""
    x = np.asarray(x, dtype=np.float32)
    tw = np.asarray(top_weights, dtype=np.float32)
    te = np.asarray(top_experts).astype(np.int64)
    B, S, H = x.shape
    T = B * S
    xf = x.reshape(T, H)

    idxs, combine = [], []
    for e in range(N_EXPERTS):
        sel = te == e                      # [T, K]
        mask = sel.any(axis=1)
        idx = np.nonzero(mask)[0]
        w_tok = (tw * sel).sum(axis=1)     # [T]
        idxs.append(idx)
        combine.append(w_tok[idx].astype(np.float32))

    n = np.array([len(i) for i in idxs])
    order = np.argsort(-n, kind="stable")
    pairs = [(int(order[i]), int(order[N_EXPERTS - 1 - i]))
             for i in range(N_PAIRS)]
    cap_a = _roundup(max(n[a] for a, _ in pairs), 8)
    cap_b = _roundup(max(n[b] for _, b in pairs), 8)
    return xf, idxs, combine, pairs, cap_a, cap_b, (B, S, H, T)


def _xT(xf, idx, cap):
    xg = np.zeros((cap, HIDDEN), np.float32)
    xg[: len(idx)] = xf[idx]
    # xT[p, ko, t] = xg[t, ko*128+p]
    return np.ascontiguousarray(
        xg.T.reshape(KO_H, P, cap).transpose(1, 0, 2)).astype(BF16)


def _ug(w_up_e, w_gate_e, h):
    # per F-half: [fc, p(h), 2(u/g), ko, f]
    wu = w_up_e[:, h * FH:(h + 1) * FH].reshape(KO_H, P, FC_N, P)
    wg = w_gate_e[:, h * FH:(h + 1) * FH].reshape(KO_H, P, FC_N, P)
    wu = wu.transpose(2, 1, 0, 3)
    wg = wg.transpose(2, 1, 0, 3)
    return np.ascontiguousarray(
        np.stack([wu, wg], axis=2)).astype(BF16)


def _wd(w_down_e, h):
    wd = w_down_e[h * FH:(h + 1) * FH, :].reshape(KO_F, P, HC_N, P)
    return np.ascontiguousarray(wd.transpose(2, 1, 0, 3)).astype(BF16)


def make_in_maps(inputs_or_none=None, *, xf, idxs, pairs, cap_a, cap_b,
                 w_up, w_gate, w_down):
    in_maps = []
    for a, b in pairs:
        xa = _xT(xf, idxs[a], cap_a)
        xb = _xT(xf, idxs[b], cap_b)
        for h in (0, 1):
            in_maps.append({
                "xa": xa,
                "xb": xb,
                "uga": _ug(w_up[a], w_gate[a], h),
                "ugb": _ug(w_up[b], w_gate[b], h),
                "wda": _wd(w_down[a], h),
                "wdb": _wd(w_down[b], h),
            })
    return in_maps


def kernel(x, weights, top_weights, top_experts, w_up, w_gate, w_down):
    w_up = np.asarray(w_up, dtype=np.float32)
    w_gate = np.asarray(w_gate, dtype=np.float32)
    w_down = np.asarray(w_down, dtype=np.float32)

    xf, idxs, combine, pairs, cap_a, cap_b, (B, S, H, T) = _prepare(
        x, top_weights, top_experts)

    in_maps = make_in_maps(xf=xf, idxs=idxs, pairs=pairs, cap_a=cap_a,
                           cap_b=cap_b, w_up=w_up, w_gate=w_gate,
                           w_down=w_down)

    key = (cap_a, cap_b)
    if key not in _compiled:
        _compiled[key] = _build(cap_a, cap_b)
    nc = _compiled[key]
    res = bass_utils.run_bass_kernel_spmd(
        nc, in_maps, core_ids=list(range(N_EXPERTS)))

    out = np.zeros((T, H), np.float32)
    for p, (a, b) in enumerate(pairs):
        r0, r1 = res.results[2 * p], res.results[2 * p + 1]
        for tag, e, cap in (("ya", a, cap_a), ("yb", b, cap_b)):
            y = (r0[tag].astype(np.float32) + r1[tag].astype(np.float32))
            y = y.reshape(H, cap)
            idx = idxs[e]
            out[idx] += y[:, : len(idx)].T * combine[e][:, None]
    return out.reshape(B, S, H)


# revision 16
# speedup vs baseline: 1.0130x; 1.0130x over previous
"""Expert-parallel MoE (DBRX-style SwiGLU FFN) on 8 TRN2 NeuronCores.

Strategy: experts are paired by token count (biggest with smallest) and each
pair is split across 2 cores along the FFN (F) axis: core 2p takes F-rows
[0:1024) of both experts in pair p, core 2p+1 takes F-rows [1024:2048).
Each core therefore runs TWO half-width SwiGLU FFNs (slot A = the pair's
bigger expert, slot B = the smaller), and the host sums the two cores'
partial y contributions. This balances PE work across cores at zero extra
weight traffic: per-core tokens go from max_e n_e (503 for seed 0) to
~(n_a+n_b)/2 (~492).

Per slot, in "transposed activation" form (all bf16, f32 PSUM):

    up^T   = w_up^T  @ x^T    (K = H = 1024, 8 K-subtiles)
    gate^T = w_gate^T @ x^T
    h^T    = silu(up^T) * gate^T
    y^T    = w_down^T @ h^T   (K = F/2 = 1024, 8 K-subtiles)

Schedule notes (from NTFF profile analysis):
  - ~6.2 us framework preamble before any user instruction.
  - DMA kicks are issued from two HWDGE engines in parallel: weights on
    Sync, x / outputs on Scalar, so first data lands ~2 us earlier than
    a single-engine kick chain.
  - up+gate weights for one 128-row F-chunk live in ONE dram tensor,
    kicked as two half DMAs so the up-half (needed first) lands sooner.
  - Junk matmuls cover preamble->first-data and ungate the HAM PE clock
    (1.2 -> 2.4 GHz after ~3.4 us of sustained activity).
  - The final output chunk is split fine (…/128/64/32) so the last
    PSUM-drain + DMA-out chain after the very last matmul is short.
"""

import numpy as np
import ml_dtypes

import concourse.bacc as bacc
import concourse.mybir as mybir
import concourse.tile as tile
from concourse import bass_utils

HIDDEN = 1024
FFN = 2048
FH = FFN // 2        # F-half per core
N_EXPERTS = 8
N_PAIRS = 4
P = 128
KO_H = HIDDEN // P   # 8  K-subtiles for up/gate (K = H)
FC_N = FH // P       # 8  F-chunks per slot (output partition tiles of stage A)
KO_F = FH // P       # 8  K-subtiles for down (K = F-half)
HC_N = HIDDEN // P   # 8  H-chunks (output partition tiles of stage B)

WARM_N = 64          # junk matmuls (N=128) covering preamble->first-data:
                     # ~32 run cold (107 ns) until HAM ungates (free-running
                     # window: fires 3.4-6.8 us after sustained PE activity
                     # begins), the rest warm (56 ns); sized so junk ends
                     # ~12.2-13.9 us, when the critical DMA prefix
                     # (xa half 0 + fc0 weights) has landed

BF16 = ml_dtypes.bfloat16

_compiled = {}  # (cap_a, cap_b) -> compiled Bacc module


def _tail_split(nt):
    """Halve the final token chunk once: the last drain+DMA chain after the
    very last matmul is short, without serializing several kick rounds."""
    if nt <= 256:
        return [nt]
    return [256, nt - 256]


def _build(cap_a, cap_b):
    f32 = mybir.dt.float32
    bf16 = mybir.dt.bfloat16

    nc = bacc.Bacc("TRN2", debug=False, enable_asserts=False,
                   num_devices=N_EXPERTS)
    xa_d = nc.dram_tensor("xa", [P, KO_H, cap_a], bf16, kind="ExternalInput")
    xb_d = nc.dram_tensor("xb", [P, KO_H, cap_b], bf16, kind="ExternalInput")
    uga_d = nc.dram_tensor("uga", [FC_N, P, 2, KO_H, P], bf16,
                           kind="ExternalInput")
    ugb_d = nc.dram_tensor("ugb", [FC_N, P, 2, KO_H, P], bf16,
                           kind="ExternalInput")
    wda_d = nc.dram_tensor("wda", [HC_N, P, KO_F, P], bf16,
                           kind="ExternalInput")
    wdb_d = nc.dram_tensor("wdb", [HC_N, P, KO_F, P], bf16,
                           kind="ExternalInput")
    ya_d = nc.dram_tensor("ya", [HC_N, P, cap_a], bf16, kind="ExternalOutput")
    yb_d = nc.dram_tensor("yb", [HC_N, P, cap_b], bf16, kind="ExternalOutput")

    with tile.TileContext(nc) as tc:
        with (
            tc.tile_pool(name="persist", bufs=1) as persist,
            tc.tile_pool(name="wpool", bufs=6) as wpool,
            tc.tile_pool(name="spool", bufs=4) as spool,
            tc.tile_pool(name="psum", bufs=2, space="PSUM") as psum,
        ):
            # PE clock warm-up: junk matmuls keep the PE busy from the end
            # of the engine preamble until the first real inputs land.
            warm = persist.tile([P, P], bf16, tag="warm")
            nc.vector.memset(warm[:], 0)
            pwarm = psum.tile([P, 512], f32, tag="pwarm", name="pwarm")
            for _ in range(WARM_N):
                nc.tensor.matmul(pwarm[:, :P], warm, warm, start=True,
                                 stop=True)

            xa = persist.tile([P, KO_H, cap_a], bf16, tag="xa")
            xb = persist.tile([P, KO_H, cap_b], bf16, tag="xb")
            hta = persist.tile([P, KO_F, cap_a], bf16, tag="hta")
            htb = persist.tile([P, KO_F, cap_b], bf16, tag="htb")

            # Early DMA bandwidth is zero-sum across queues: the first real
            # matmul chain needs exactly [xa ko0-3, ug0-up, ug0-gate], so
            # those transfers go SERIALLY FIRST on Sync's queue (fat 4032 B/
            # partition halves transfer faster than 2016 B quarters), then
            # the second xa half. xb is kicked from Scalar's queue
            # mid-stage-A (emitted in the fc loop below) when the critical
            # window has passed.
            nc.sync.dma_start(xa[:, 0:4], xa_d.ap()[:, 0:4])

            slots = (
                (xa, hta, uga_d, wda_d, ya_d, cap_a),
                (xb, htb, ugb_d, wdb_d, yb_d, cap_b),
            )
            for s, (x_t, ht, ug_d, wd_d, y_d, cap) in enumerate(slots):
                tchunks = [(t0, min(512, cap - t0))
                           for t0 in range(0, cap, 512)]

                # Stage A: h^T[fc] = silu(up^T) * gate^T per 128-row F-chunk
                for fc in range(FC_N):
                    ug_t = wpool.tile([P, 2, KO_H, P], bf16, tag="ug")
                    # up half first (pu chain leads), then gate half
                    nc.sync.dma_start(ug_t[:, 0], ug_d.ap()[fc][:, 0])
                    nc.sync.dma_start(ug_t[:, 1], ug_d.ap()[fc][:, 1])
                    if s == 0 and fc == 0:
                        # rest of xa, behind the fc0 weights on the same queue
                        nc.sync.dma_start(xa[:, 4:8], xa_d.ap()[:, 4:8])
                    for t0, nt in tchunks:
                        pu = psum.tile([P, 512], f32, tag="pu",
                                       name="pu")[:, :nt]
                        pg = psum.tile([P, 512], f32, tag="pg",
                                       name="pg")[:, :nt]
                        # interleave the two accumulation chains, pu leading
                        # pg by one ko: x quarters and the up weight half are
                        # consumed as soon as they land
                        seq = []
                        for ko in range(KO_H):
                            seq.append((pu, 0, ko))
                            if ko >= 1:
                                seq.append((pg, 1, ko - 1))
                        seq.append((pg, 1, KO_H - 1))
                        for dst, ug_i, ko in seq:
                            nc.tensor.matmul(dst, ug_t[:, ug_i, ko],
                                             x_t[:, ko, t0:t0 + nt],
                                             start=(ko == 0),
                                             stop=(ko == KO_H - 1))
                        su = spool.tile([P, 512], f32, tag="silu",
                                        name="su")[:, :nt]
                        nc.scalar.activation(
                            su, pu, mybir.ActivationFunctionType.Sigmoid)
                        nc.vector.tensor_mul(su, su, pu)
                        nc.vector.tensor_mul(ht[:, fc, t0:t0 + nt], su, pg)
                    if s == 0 and fc == 1:
                        # xb streams on Scalar's queue once the early window
                        # is past; needed only at slot B (~40 us later). The
                        # dummy 1-col copy creates a data dependency on fc1's
                        # output so the scheduler cannot hoist the 1 MB xb
                        # transfer into the critical first-weights window
                        # (it would starve that stream of DMA bandwidth).
                        nc.vector.tensor_copy(xb[:, 0, 0:1], ht[:, 1, 0:1])
                        nc.scalar.dma_start(xb[:], xb_d.ap())

                # Stage B: y^T[hc] = w_down^T @ h^T per 128-row H-chunk
                for hc in range(HC_N):
                    wd_t = wpool.tile([P, KO_F, P], bf16, tag="wd")
                    nc.sync.dma_start(wd_t[:], wd_d.ap()[hc])
                    chunks = tchunks
                    if s == len(slots) - 1 and hc == HC_N - 1:
                        t0l, ntl = tchunks[-1]
                        chunks = list(tchunks[:-1])
                        for sz in _tail_split(ntl):
                            chunks.append((t0l, sz))
                            t0l += sz
                    for ci, (t0, nt) in enumerate(chunks):
                        py = psum.tile([P, 512], f32, tag="py",
                                       name="py")[:, :nt]
                        for ko in range(KO_F):
                            nc.tensor.matmul(py, wd_t[:, ko],
                                             ht[:, ko, t0:t0 + nt],
                                             start=(ko == 0),
                                             stop=(ko == KO_F - 1))
                        yo = spool.tile([P, 512], bf16, tag="yo",
                                        name="yo")[:, :nt]
                        nc.vector.tensor_copy(yo, py)
                        # the very last chunks alternate kick engines so the
                        # final two output DMAs issue in parallel, not FIFO
                        last_hc = (s == len(slots) - 1 and hc == HC_N - 1)
                        eng = nc.sync if (last_hc and ci == len(chunks) - 1) \
                            else nc.scalar
                        eng.dma_start(y_d.ap()[hc][:, t0:t0 + nt], yo)

    nc.compile()
    return nc


def _roundup(v, m):
    return max(((v + m - 1) // m) * m, 64)


def _prepare(x, top_weights, top_experts):
    """Host routing: unique tokens + combine weight per expert, pairing."""
    x = np.asarray(x, dtype=np.float32)
    tw = np.asarray(top_weights, dtype=np.float32)
    te = np.asarray(top_experts).astype(np.int64)
    B, S, H = x.shape
    T = B * S
    xf = x.reshape(T, H)

    idxs, combine = [], []
    for e in range(N_EXPERTS):
        sel = te == e                      # [T, K]
        mask = sel.any(axis=1)
        idx = np.nonzero(mask)[0]
        w_tok = (tw * sel).sum(axis=1)     # [T]
        idxs.append(idx)
        combine.append(w_tok[idx].astype(np.float32))

    n = np.array([len(i) for i in idxs])
    order = np.argsort(-n, kind="stable")
    pairs = [(int(order[i]), int(order[N_EXPERTS - 1 - i]))
             for i in range(N_PAIRS)]
    cap_a = _roundup(max(n[a] for a, _ in pairs), 8)
    cap_b = _roundup(max(n[b] for _, b in pairs), 8)
    return xf, idxs, combine, pairs, cap_a, cap_b, (B, S, H, T)


def _xT(xf, idx, cap):
    xg = np.zeros((cap, HIDDEN), np.float32)
    xg[: len(idx)] = xf[idx]
    # xT[p, ko, t] = xg[t, ko*128+p]
    return np.ascontiguousarray(
        xg.T.reshape(KO_H, P, cap).transpose(1, 0, 2)).astype(BF16)


def _ug(w_up_e, w_gate_e, h):
    # per F-half: [fc, p(h), 2(u/g), ko, f]
    wu = w_up_e[:, h * FH:(h + 1) * FH].reshape(KO_H, P, FC_N, P)
    wg = w_gate_e[:, h * FH:(h + 1) * FH].reshape(KO_H, P, FC_N, P)
    wu = wu.transpose(2, 1, 0, 3)
    wg = wg.transpose(2, 1, 0, 3)
    return np.ascontiguousarray(
        np.stack([wu, wg], axis=2)).astype(BF16)


def _wd(w_down_e, h):
    wd = w_down_e[h * FH:(h + 1) * FH, :].reshape(KO_F, P, HC_N, P)
    return np.ascontiguousarray(wd.transpose(2, 1, 0, 3)).astype(BF16)


def make_in_maps(inputs_or_none=None, *, xf, idxs, pairs, cap_a, cap_b,
                 w_up, w_gate, w_down):
    in_maps = []
    for a, b in pairs:
        xa = _xT(xf, idxs[a], cap_a)
        xb = _xT(xf, idxs[b], cap_b)
        for h in (0, 1):
            in_maps.append({
                "xa": xa,
                "xb": xb,
                "uga": _ug(w_up[a], w_gate[a], h),
                "ugb": _ug(w_up[b], w_gate[b], h),
                "wda": _wd(w_down[a], h),
                "wdb": _wd(w_down[b], h),
            })
    return in_maps


def kernel(x, weights, top_weights, top_experts, w_up, w_gate, w_down):
    w_up = np.asarray(w_up, dtype=np.float32)
    w_gate = np.asarray(w_gate, dtype=np.float32)
    w_down = np.asarray(w_down, dtype=np.float32)

    xf, idxs, combine, pairs, cap_a, cap_b, (B, S, H, T) = _prepare(
        x, top_weights, top_experts)

    in_maps = make_in_maps(xf=xf, idxs=idxs, pairs=pairs, cap_a=cap_a,
                           cap_b=cap_b, w_up=w_up, w_gate=w_gate,
                           w_down=w_down)

    key = (cap_a, cap_b)
    if key not in _compiled:
        _compiled[key] = _build(cap_a, cap_b)
    nc = _compiled[key]
    res = bass_utils.run_bass_kernel_spmd(
        nc, in_maps, core_ids=list(range(N_EXPERTS)))

    out = np.zeros((T, H), np.float32)
    for p, (a, b) in enumerate(pairs):
        r0, r1 = res.results[2 * p], res.results[2 * p + 1]
        for tag, e, cap in (("ya", a, cap_a), ("yb", b, cap_b)):
            y = (r0[tag].astype(np.float32) + r1[tag].astype(np.float32))
            y = y.reshape(H, cap)
            idx = idxs[e]
            out[idx] += y[:, : len(idx)].T * combine[e][:, None]
    return out.reshape(B, S, H)
